# revision 1
# baseline (speedup 1.0000x reference)
"""Trainium2 Bass kernel for MiniEq2Net (gnn_message_passing).

Math (validated against the jax reference in float64, rel err ~3e-7):

Per batch b (X = x[b], [n=256, d=16]) the first eq-layer's input channels are
diag(X[:,d]) and X[:,d] outer X[:,d], so layer 1 collapses to
    G1[s] = S(s) + c'_{s,i} (row-broadcast) + delta_ij a_{s,i}
with S(s) = X diag(wt_s) X^T (symmetric, one K=64 matmul per 4-row group in a
packed (a=i%4, s) x (j) layout), and the diagonal handled exactly via tiny
[32,256] side computations (dn/dg/Hdc).  Layer 2 + pooling becomes two K=128
block-diagonal channel-mix matmuls over relu'd H and H^T plus a fused
relu-accumulate, with the diagonal / rowsum / total-sum basis terms folded
into per-partition biases and a closed-form correction.

Sharding: pure data parallel, one batch element per NeuronCore (B=8, 8 cores).

v3 layout: phase A runs two INDEPENDENT PSUM streams -- the H stream in
1-bank [128,512] units (2 groups: K=64 S-matmul per group + per-group
bias-relu with r4 accumulation on DVE/Pool) and the HT stream in 2-bank
[128,1024] tiles (K=96 matmuls whose extra 32 rows are a replicated
identity against a cp-bias rhs, folding the c'_j add into the matmul; one
wide relu per tile on Act).  Per-group lhsT tiles are built by 94ns bf16
4x-mode tensor_scalar ops on DVE (rows 64-95 DMA'd once from a blob).
Phase B: 2 groups per PSUM bank, relus greedy-balanced across the three
elementwise engines; pooling + the tiny MLP head run on the host from the
DMA'd-out acc tile (col 64 carries the diagonal correction).
"""

import numpy as np

N = 256          # n (graph nodes)
D = 16           # input channel count
NH = 32          # hidden channels
A = 4            # row-packing factor: partition p = a*32+s, row i = 4*g+a
G = N // A       # 64 row-groups
B = 8            # batch == cores
ST = 4           # groups per phase-A super-tile (2 PSUM banks)
NST = G // ST    # number of super-tiles
F32 = np.float32

_PROG_CACHE = {}


def _reorder_ag(arr):
    """Permute the trailing i axis (len 256) into (a, g) order:
    out[..., a*G+g] = arr[..., 4*g+a]."""
    sh = arr.shape[:-1]
    return arr.reshape(*sh, G, A).swapaxes(-1, -2).reshape(*sh, N)


# Blob packing: blob_name -> (partition_count, [(tensor_name, P, F), ...])
_BLOBS = {
    'blobcp': (128, [
        ('Cpp', 128, G), ('hdc4', 128, G),
    ]),
    'blob128': (128, [
        ('WB0', 128, 128), ('WB1', 128, 128),
        ('WB3', 128, 128), ('PWrep', 128, 128), ('b2rep', 128, 1),
    ]),
    'blob64': (64, [
        ('Xr', 64, G),
    ]),
    'blob32': (32, [
        ('qsb', 32, 256), ('u2sb', 32, 256),
    ]),
}
# bf16 blobs: WtBDh feeds the lhsT prep; I32rep is DMA'd once into
# partitions 64-95 of lhsT_all (the K=96 HT-matmul's constant rows).
_BLOBS_BF = {
    'blobbf': (64, [
        ('WtBDh', 64, 128), ('XT4h', 64, 256),
    ]),
    'blob96': (96, [
        ('rhs96', 96, 256),
    ]),
    'blobi32': (32, [
        ('I32rep', 32, G * 128),
    ]),
}


def _blob_layout():
    where, shapes = {}, {}
    for src in (_BLOBS, _BLOBS_BF):
        for bname, (pb, items) in src.items():
            off = 0
            for tname, p, f in items:
                where[tname] = (bname, p, off, f)
                off += f
            shapes[bname] = (pb, off)
    return where, shapes


_WHERE, _BLOB_SHAPES = _blob_layout()


# ---------------------------------------------------------------- host side

def _percore_inputs(xb, W1, b1, W2, b2, D1, db1, D2, db2, D3, db3):
    """Small per-core operands, precomputed in float64, packed into blobs."""
    import ml_dtypes
    X = xb.astype(np.float64)                      # [256, 16]
    n = float(N)
    sigma = X.sum(0)
    wt = W1[D:, :, 0] + W1[D:, :, 1]               # [16,32]
    alpha = W1[:D, :, 0] + W1[:D, :, 1] + W1[:D, :, 2]
    beta = W1[D:, :, 2]
    abias = alpha.T @ X.T + beta.T @ (X.T ** 2)    # [32,256]
    gamma = W1[:D, :, 3] / n + W1[D:, :, 3] * sigma[:, None] / n
    k = (W1[:D, :, 4].T @ (sigma / n**2)
         + W1[D:, :, 4].T @ (sigma**2 / n**2) + b1)
    cp = gamma.T @ X.T + k[:, None]                # [32,256]
    XT = X.T

    WtBD = np.zeros((A * D, 128))
    for a in range(A):
        WtBD[a * D:(a + 1) * D, a * NH:(a + 1) * NH] = wt
    Xr = X.reshape(G, A, D).transpose(1, 2, 0).reshape(A * D, G)
    Cpp = cp.reshape(NH, G, A).transpose(2, 0, 1).reshape(128, G)

    def blockdiag(M):
        out = np.zeros((128, 128))
        for a in range(A):
            out[a * NH:(a + 1) * NH, a * NH:(a + 1) * NH] = M
        return out

    I32r4 = np.tile(np.eye(NH), (1, A))
    # diagonal-channel side computations, all host-side ((a, g) col order):
    # dn/dg are the relu'd diagonal entries without/with the diag-embed
    # contribution; hdc corrects r4's missing diagonal, qsb/u2sb feed the
    # on-device closed-form correction.
    t0 = wt.T @ _reorder_ag(XT ** 2) + _reorder_ag(cp)     # [32, 256]
    dn = np.maximum(t0, 0.0)
    dg = np.maximum(t0 + _reorder_ag(abias), 0.0)
    hdc = dg - dn
    hdc4 = hdc.reshape(NH, A, G).transpose(1, 0, 2).reshape(128, G)
    W01 = W2[:, :, 0] + W2[:, :, 1]
    vals = {
        'XT4h': np.tile(XT, (A, 1)),
        'Xr': Xr,
        'rhs96': np.concatenate([np.tile(XT, (A, 1)), cp], axis=0),
        'I32rep': np.tile(I32r4, (1, G)),
        'Cpp': Cpp, 'hdc4': hdc4,
        'WB0': blockdiag(W2[:, :, 0]), 'WB1': blockdiag(W2[:, :, 1]),
        'WB3': blockdiag(W2[:, :, 3] / n),
        'PWrep': (np.tile(np.eye(NH), (A, 1)) @ (W2[:, :, 4] / n**2)) @ I32r4,
        'qsb': W01.T @ hdc + W2[:, :, 2].T @ dg,
        'u2sb': W01.T @ dn,
        'b2rep': np.tile(b2, A)[:, None],
        'WtBDh': WtBD,
    }
    blobs = {}
    for bn, sh in _BLOB_SHAPES.items():
        dt = ml_dtypes.bfloat16 if bn in _BLOBS_BF else F32
        blobs[bn] = np.zeros(sh, dtype=dt)
    for tname, (bn, p, off, f) in _WHERE.items():
        v = np.asarray(vals[tname], dtype=np.float64)
        assert v.shape == (p, f), (tname, v.shape, (p, f))
        blobs[bn][0:p, off:off + f] = v.astype(blobs[bn].dtype)
    return blobs


# -------------------------------------------------------------- device side

def build_program():
    if 'nc' in _PROG_CACHE:
        return _PROG_CACHE['nc']

    from contextlib import ExitStack
    import concourse.bacc as bacc
    import concourse.tile as tile
    from concourse import mybir

    f32 = mybir.dt.float32
    f32r = mybir.dt.float32r
    bf16 = mybir.dt.bfloat16
    AF = mybir.ActivationFunctionType
    ALU = mybir.AluOpType

    nc = bacc.Bacc(trn_type="TRN2", target_bir_lowering=False)
    dram = {}
    for bn, sh in _BLOB_SHAPES.items():
        dt = bf16 if bn in _BLOBS_BF else f32
        dram[bn] = nc.dram_tensor(bn, list(sh), dt, kind="ExternalInput")
    yout_d = nc.dram_tensor("yout", [128, G + 1], f32, kind="ExternalOutput")

    with tile.TileContext(nc) as tc:
        ctx = ExitStack()
        consts = ctx.enter_context(tc.tile_pool(name="consts", bufs=1))
        # DMA issue order (single queue; HWDGE serializes anyway): blob64
        # (Xr for preps + XT4) and blobbf (WtBDh) gate phase A's start.
        big = ctx.enter_context(tc.tile_pool(name="big", bufs=1))
        zero256 = big.tile([128, 256], f32, name="zero256")
        nc.vector.memset(zero256, 0.0)
        H4 = big.tile([128, G * N], f32r, name="H4")
        HT4 = big.tile([128, G * N], f32r, name="HT4")
        r4 = big.tile([128, G], f32, name="r4")
        # col 64 carries the diagonal correction; the whole tile is DMA'd
        # out at the end (pooling + MLP head run on the host)
        acc = big.tile([128, G + 1], f32, name="acc")
        nc.vector.memset(acc[:, G:G + 1], 0.0)
        # Per-group lhsT tiles: rows 0-63 = WtBD * Xr[:, g] (prep TSP ops,
        # K=64 H-matmul); rows 64-95 = replicated identity (constant, DMA'd
        # once) extending the same slices to K=96 for the HT-matmul, whose
        # rhs rows 64-95 carry the c'_j bias table.
        lhsT_all = big.tile([96, G, 128], bf16, name="lhsT_all")

        bt = {}
        # blobi32 first: it writes into lhsT_all, and the dep tracker gates
        # every S-matmul (same tile) on that DMA's completion.
        for bn in ('blobi32', 'blob64', 'blobbf', 'blobcp', 'blob96',
                   'blob32', 'blob128'):
            if bn == 'blobi32':
                nc.default_dma_engine.dma_start(out=lhsT_all[64:96, :, :],
                                                in_=dram[bn].ap())
                continue
            sh = _BLOB_SHAPES[bn]
            dt = bf16 if bn in _BLOBS_BF else f32
            t = consts.tile(list(sh), dt, name=f"sb_{bn}")
            nc.default_dma_engine.dma_start(out=t, in_=dram[bn].ap())
            bt[bn] = t
        sb = {tn: bt[bn][0:p, off:off + f]
              for tn, (bn, p, off, f) in _WHERE.items() if bn in bt}

        scrap_pool = ctx.enter_context(tc.tile_pool(name="scrap", bufs=6))
        small = ctx.enter_context(tc.tile_pool(name="small", bufs=1))

        # bf16 rhs for the bf16-lhsT matmuls (the backend rejects mixed
        # f32r x bf16); fp32r bitcasts elsewhere avoid conversion copies.
        xt4r = sb['XT4h']
        rhs96r = sb['rhs96']
        wb0r = consts.tile([128, 128], f32r, name="wb0r")
        wb1r = consts.tile([128, 128], f32r, name="wb1r")

        # ---- lhsT prep: lhsT_all[k, g, m] = WtBD[k, m] * Xr[k, g].
        # Per-group tensor_scalar: bf16 in/out + fp32 per-partition scalar
        # hits the DVE 4x perf mode (93ns/op).  First super-tile's groups up
        # front; the rest interleave into the phase-A loop.
        def prep(g):
            # on Pool: gpsimd may not touch PSUM, so it owns all SBUF-side
            # prep work while DVE/Act carry the PSUM relus
            nc.gpsimd.tensor_scalar(lhsT_all[0:64, g, :], sb['WtBDh'],
                                    sb['Xr'][:, g:g + 1], None, ALU.mult)

        PREP_AHEAD = 6  # H-units of lookahead in DVE's program order
        for g in range(PREP_AHEAD * 2):
            prep(g)

        psA_ctx = ExitStack()
        psH_pool = psA_ctx.enter_context(
            tc.tile_pool(name="psH", bufs=4, space="PSUM"))
        psHT_pool = psA_ctx.enter_context(
            tc.tile_pool(name="psHT", bufs=2, space="PSUM"))

        # ---- Phase A: 2-bank super-tiles of 4 groups, H and HT streams
        # fully independent (HT's c'_j bias rides the K=96 matmul).
        # DVE: 2 H-relus + prep lookahead; Pool: 2 H-relus; Act: the whole
        # HT relu (no accum -> no 187ns read-accumulator penalty).

        def relu_bias_acc(eng, out, in_, bias, accum):
            # (in + bias) relu, accum_out = rowsum; tensor_scalar can't fuse
            # two scalar stages with accum, so D/P use scalar_tensor_tensor.
            if eng == 'A':
                nc.scalar.activation(out=out, in_=in_, func=AF.Relu,
                                     bias=bias, accum_out=accum)
            else:
                e = nc.vector if eng == 'D' else nc.gpsimd
                e.scalar_tensor_tensor(out, in_, bias, zero256,
                                       ALU.add, ALU.max, accum_out=accum)

        def relu_plain(eng, out, in_):
            if eng == 'A':
                nc.scalar.activation(out=out, in_=in_, func=AF.Relu)
            else:
                e = nc.vector if eng == 'D' else nc.gpsimd
                e.tensor_scalar(out, in_, 0.0, None, ALU.max)

        # PE warm-up: dummy matmuls while the blob DMAs land, so the
        # tensor engine's pstate ramp completes before the first real work
        # (the tile has no readers, so its pool slot frees immediately).
        psW = psH_pool.tile([128, 512], f32, name="psH")
        for w in range(8):
            nc.tensor.matmul(psW[:, 256:384], lhsT=zero256[0:64, 0:128],
                             rhs=zero256[0:64, 0:128], start=True, stop=True,
                             skip_group_check=True)
        # phase-B fp32r operands: the backend requires a real rounding
        # producer for f32r matmul inputs, so convert via tensor_copy
        nc.gpsimd.tensor_copy(wb0r, sb['WB0'])
        nc.gpsimd.tensor_copy(wb1r, sb['WB1'])

        # H stream: 1-bank units of 2 groups (bufs=4 -> deep pipelining);
        # HT stream: 2-bank tiles of 4 groups, relu'd wide on Act.
        for u in range(G // 2):
            psh = psH_pool.tile([128, 512], f32, name="psH")
            for j in range(2):
                g = 2 * u + j
                nc.tensor.matmul(psh[:, j * N:(j + 1) * N],
                                 lhsT=lhsT_all[0:64, g, :], rhs=xt4r,
                                 start=(j == 0), stop=(j == 1),
                                 skip_group_check=True)
            if (u + PREP_AHEAD) * 2 < G:
                prep(2 * (u + PREP_AHEAD))
                prep(2 * (u + PREP_AHEAD) + 1)
            for j in range(2):
                g = 2 * u + j
                eng = 'A' if (j == 0 and u % 4 == 3) else 'D'
                relu_bias_acc(eng, H4[:, g * N:(g + 1) * N],
                              psh[:, j * N:(j + 1) * N],
                              sb['Cpp'][:, g:g + 1], r4[:, g:g + 1])
            if u % 2 == 1:
                g0 = (u - 1) * 2
                psht = psHT_pool.tile([128, ST * N], f32, name="psHT")
                for j in range(ST):
                    g = g0 + j
                    nc.tensor.matmul(psht[:, j * N:(j + 1) * N],
                                     lhsT=lhsT_all[0:96, g, :], rhs=rhs96r,
                                     start=(j % 2 == 0), stop=(j % 2 == 1),
                                     skip_group_check=True)
                relu_plain('A', HT4[:, g0 * N:(g0 + ST) * N], psht)

        # ---- Small-phase suffix: rho/kappa biases (needs all of r4)
        # suffix PSUM rides one psHT-pool tile (banks: PW/krep in bank 0,
        # WB3 in bank 1) so most banks drain early for phase B's first tiles
        psT = psHT_pool.tile([128, ST * N], f32, name="psHT")
        psT2 = psT[:, 512:512 + G]
        r4hat = small.tile([128, G], f32, name="r4hat")
        nc.vector.tensor_add(r4hat, r4, sb['hdc4'])
        rsum = small.tile([128, 1], f32, name="rsum")
        nc.vector.tensor_reduce(out=rsum, in_=r4hat,
                                axis=mybir.AxisListType.X, op=ALU.add)
        # PWrep = PW @ I32r4 (host-folded), so one matmul yields the
        # already-replicated kappa and the old replicate+copy hops vanish
        nc.tensor.matmul(psT[:, 4:5], lhsT=sb['PWrep'], rhs=rsum,
                         start=True, stop=True, skip_group_check=True)
        krep = small.tile([128, 1], f32, name="krep")
        nc.scalar.activation(out=krep, in_=psT[:, 4:5], func=AF.Identity,
                             bias=sb['b2rep'])
        nc.tensor.matmul(psT2, lhsT=sb['WB3'], rhs=r4hat,
                         start=True, stop=True, skip_group_check=True)
        rhoka = small.tile([128, G], f32, name="rhoka")
        nc.scalar.activation(out=rhoka, in_=psT2,
                             func=AF.Identity, bias=krep)
        psA_ctx.close()

        # corr path ((a,g) order throughout) — runs parallel with phase B
        rhokr = small.tile([32, 256], f32, name="rhokr")
        for a in range(A):
            nc.default_dma_engine.dma_start(
                out=rhokr[:, a * G:(a + 1) * G],
                in_=rhoka[a * NH:(a + 1) * NH, :])
        uii = small.tile([32, 256], f32, name="uii")
        nc.gpsimd.tensor_add(uii, sb['u2sb'], rhokr)
        t3 = small.tile([32, 256], f32, name="t3")
        nc.gpsimd.tensor_add(t3, uii, sb['qsb'])
        scrapS = small.tile([32, 256], f32, name="scrapS")
        cA = small.tile([32, 1], f32, name="cA")
        nc.vector.tensor_scalar(scrapS, t3, 0.0, None, ALU.max, ALU.add,
                                accum_out=cA)
        scrapS2 = small.tile([32, 256], f32, name="scrapS2")
        cB = small.tile([32, 1], f32, name="cB")
        nc.vector.tensor_scalar(scrapS2, uii, 0.0, None, ALU.max, ALU.add,
                                accum_out=cB)
        nc.vector.tensor_sub(acc[0:32, G:G + 1], cA, cB)

        # ---- Phase B: channel mix + fused bias-relu-rowsum.
        # 2 groups per PSUM bank; relu engines rotate.
        # Engine balance for the 64 B-relus (D 392 / A 585 / P 451ns):
        # greedy by load, but Act-heavy early and DVE/Pool-only for the last
        # groups so the relu wave doesn't trail the final matmuls on the
        # slow engine.
        B_COST = {'D': 392, 'A': 585}
        b_load = {'D': 0, 'A': 0}
        B_ENG = []
        for g in range(G):
            cand = 'D' if g >= G - 3 else 'DA'
            e = min(cand, key=lambda k: b_load[k] + B_COST[k])
            b_load[e] += B_COST[e]
            B_ENG.append(e)
        psU_pool = ctx.enter_context(
            tc.tile_pool(name="psU", bufs=4, space="PSUM"))
        for c in range(G // 2):
            ps = psU_pool.tile([128, 512], f32, name="psU")
            for k in range(2):
                g = 2 * c + k
                sl = slice(g * N, (g + 1) * N)
                half = ps[:, k * N:(k + 1) * N]
                nc.tensor.matmul(half, lhsT=wb0r, rhs=H4[:, sl],
                                 start=(k == 0), stop=False,
                                 skip_group_check=True)
                nc.tensor.matmul(half, lhsT=wb1r, rhs=HT4[:, sl],
                                 start=False, stop=(k == 1),
                                 skip_group_check=True)
            for k in range(2):
                g = 2 * c + k
                scrap = scrap_pool.tile([128, N], f32, name="scrap")
                relu_bias_acc(B_ENG[g], scrap,
                              ps[:, k * N:(k + 1) * N],
                              rhoka[:, g:g + 1], acc[:, g:g + 1])

        # ---- acc (+corr in col 64) out; pooling + MLP head run on host
        nc.default_dma_engine.dma_start(out=yout_d.ap(), in_=acc)

        ctx.close()

    nc.compile()
    _PROG_CACHE['nc'] = nc
    return nc


def make_in_maps(inputs):
    x = np.asarray(inputs['x'], dtype=F32)
    args = [np.asarray(inputs[k], dtype=np.float64) for k in
            ('W1', 'b1', 'W2', 'b2', 'D1', 'db1', 'D2', 'db2', 'D3', 'db3')]
    return [_percore_inputs(x[b], *args) for b in range(B)]


def finish_host(out, inputs):
    """Pooling + tiny MLP head on the host: out is the device's [128, G+1]
    acc tile (col G = diagonal correction, rows 0:32)."""
    accred = out[:, 0:G].astype(np.float64).sum(1)          # [128]
    corr = out[0:32, G].astype(np.float64)
    p = np.maximum(accred.reshape(A, NH).sum(0) + corr, 0)  # [32]
    h = np.maximum(p @ inputs['D1'] + inputs['db1'], 0)
    h = np.maximum(h @ inputs['D2'] + inputs['db2'], 0)
    return (h @ inputs['D3'] + inputs['db3']).astype(F32)


def kernel(**inputs) -> np.ndarray:
    from concourse.bass_utils import run_bass_kernel_spmd
    nc = build_program()
    in_maps = make_in_maps(inputs)
    res = run_bass_kernel_spmd(nc, in_maps, core_ids=list(range(B))).results
    return np.stack([finish_host(np.asarray(res[b]['yout']), inputs)
                     for b in range(B)], axis=0).astype(F32)

